# revision 1
# baseline (speedup 1.0000x reference)
"""Depth-weighted average pooling (3x3, stride 2) on 8 Trainium2 NeuronCores.

out[n,c,ho,wo] = sum_ij x[n,c,2ho+i,2wo+j] * w_ij / sum_ij w_ij
  w_ij = exp(-|d[n,2ho+1,2wo+1] - d[n,2ho+i,2wo+j]|)

Sharding: batch N=8, one image per core (data parallel, no halo).

Per-core layout ("rows mod 4" mapping): SBUF partition p holds input rows
4p..4p+3 (tile dim t) plus a re-read of row 4p+4 (T0'), so partition p
computes output rows 2p ("even sub", input rows 4p..4p+2) and 2p+1
("odd sub", rows 4p+2..4p+4).  All engine access patterns start at
partition 0 (hardware requires start partition in {0,32,64,96}).
Weights are computed in the same [p, sub, wo] layout, so they are direct
per-partition operands of the vector engine — no cross-partition
broadcast is ever needed.  Channels live in the free dimension.
"""

import os
import sys
import functools

import numpy as np

for _p in ("/opt/trn_rl_repo", "/opt/trn_rl_repo/concourse"):
    if os.path.isdir(_p) and _p not in sys.path:
        sys.path.insert(0, _p)

KH = KW = 3
SH = SW = 2
N_CORES = 8
C, H, W = 64, 512, 512

# defaults used by kernel() and the test harness
DEFAULT_VARIANT = "fp32"
DEFAULT_G = 4

# taps in order; (1,1) is the center
TAPS = [(i, j) for i in range(3) for j in range(3)]
NC_TAPS = [t for t in TAPS if t != (1, 1)]


def AP_load5(x, c0, G, P, W):
    """DRAM view for [p, c, 5, w] <- x[c, 4p+t, w]: 5*W contiguous per (p,c)."""
    import concourse.bass as bass

    HW_ = x.shape[1] * x.shape[2]
    return bass.AP(
        x.tensor,
        c0 * HW_,
        [[4 * W, P], [HW_, G], [1, 5 * W]],
    )


def _main_bf16(nc, tc, xp, pp, ap_, x, o, wmb, C, G, W, Ho, Wo, Wp, PE, PO, ablate=""):
    """bf16 main channel loop.

    Per group: SWDGE cast-DMA loads x rows as contiguous bf16; ScalarE
    de-interleaves each row into three 4B-aligned tap arrays
    [j0 | j1 | j2] (each Wp wide) so every vector op runs in the packed
    2x bf16 mode; the odd-sub i=2 operand is built by a partition-shift
    SBUF->SBUF DMA of the expanded tile.  Accumulation in bf16; the
    store casts back to fp32 in the DMA.
    """
    from concourse import mybir

    bf16 = mybir.dt.bfloat16
    f32 = mybir.dt.float32

    preXC = None
    if "nodma" in ablate:
        preXC = ap_.tile([PE, G, 5, W], bf16, tag="preXC")
        nc.vector.memzero(preXC[:])
    for g in range(C // G):
        c0 = g * G
        if "nodma" in ablate:
            XC = preXC
        else:
            # main load rows 4p..4p+3 (8 KB DRAM reads) + t=4 re-read,
            # cast fp32->bf16 in the DMA (SWDGE ring carries only loads)
            XC = xp.tile([PE, G, 5, W], bf16, tag="XC")
            nc.gpsimd.dma_start(
                out=XC[0:PE, :, 0:4],
                in_=x[c0 : c0 + G, 0 : 4 * PE, :].rearrange(
                    "c (p t) w -> p c (t w)", t=4
                ),
            )
            nc.gpsimd.dma_start(
                out=XC[0:PO, :, 4],
                in_=x[c0 : c0 + G, 4 : 4 * PO + 1 : 4, :].transpose([1, 0, 2]),
            )
            if PE > PO:  # t=4 pad for the last partition
                nc.gpsimd.dma_start(
                    out=XC[PO:PE, :, 4],
                    in_=x[c0 : c0 + G, 0:1, :].unsqueeze(0),
                )
        # expand rows t=0..4: XB[p,c,t] = [x[..,0::2] | x[..,1::2] | x[..,2::2] pad]
        XB = xp.tile([PE, G, 5, 3 * Wp], bf16, tag="XB")
        nc.scalar.copy(XB[0:PE, :, :, 0:Wp], XC[0:PE, :, :, 0 : 2 * Wp : 2])
        nc.scalar.copy(XB[0:PE, :, :, Wp : 2 * Wp], XC[0:PE, :, :, 1 : 2 * Wp : 2])
        nc.scalar.copy(
            XB[0:PE, :, :, 2 * Wp : 2 * Wp + Wo],
            XC[0:PE, :, :, 2 : 2 + 2 * Wo : 2],
        )
        # defined (finite) pad column for j2
        nc.scalar.copy(XB[0:PE, :, :, 3 * Wp - 1 : 3 * Wp], XC[0:PE, :, :, 0:1])

        def xv_m(i, j):  # [PE, G, 2, Wp]; sub stride = 2 slots in t
            return XB[0:PE, :, i : i + 3 : 2, j * Wp : (j + 1) * Wp]

        def wv_m(wb):
            return wb[0:PE].unsqueeze(1).broadcast_to([PE, G, 2, Wp])

        acc = ap_.tile([PE, G, 2, Wp], bf16, tag="acc")
        if "nodve" in ablate:
            nc.vector.memzero(acc[:])
        else:
            nc.vector.tensor_mul(acc[:], xv_m(1, 1), wv_m(wmb[(1, 1)]))
            for (i, j) in NC_TAPS:
                p = pp.tile([PE, G, 2, Wp], bf16, tag="pm")
                nc.vector.tensor_mul(p[:], xv_m(i, j), wv_m(wmb[(i, j)]))
                nc.vector.tensor_add(acc[:], acc[:], p[:])

        if "nodma" in ablate:
            continue
        # convert to fp32 on ScalarE, then merged 2-row store on the ACT ring
        OS = ap_.tile([PE, G, 2, Wo], f32, tag="OS")
        nc.scalar.copy(OS[:], acc[0:PE, :, :, 0:Wo])
        nc.scalar.dma_start(
            out=o[c0 : c0 + G, 0 : 2 * PO, :].rearrange(
                "c (p s) w -> p c (s w)", s=2
            ),
            in_=OS[0:PO, :, :, :],
        )
        if Ho > 2 * PO:
            nc.scalar.dma_start(
                out=o[c0 : c0 + G, 2 * PO : Ho, :].transpose([1, 0, 2]),
                in_=OS[PO:PE, :, 0, :],
            )


def build_kernel(C=C, H=H, W=W, G=4, repeat=1, variant="fp32", ablate=""):
    """Single-core Bass program: x[C,H,W], d[H,W] -> o[C,Ho,Wo].

    ablate: "nodve" skips the main-loop vector ops; "nodma" skips the
    x loads (compute reads whatever is resident).  For A/B timing only.
    """
    from contextlib import ExitStack

    import concourse.bacc as bacc
    from concourse.tile import TileContext
    from concourse import mybir

    f32 = mybir.dt.float32
    bf16 = mybir.dt.bfloat16
    AluOp = mybir.AluOpType
    Act = mybir.ActivationFunctionType
    Wp = W // 2  # padded output-width for bf16 tiles (= Wo+1, even)

    Ho = (H - KH) // SH + 1
    Wo = (W - KW) // SW + 1
    assert C % G == 0 and H % 4 == 0
    PE = (Ho + 1) // 2  # partitions carrying an even-sub output row
    PO = Ho // 2  # partitions carrying an odd-sub output row
    assert PE <= 128 and Ho == 2 * PE - 1

    nc = bacc.Bacc(
        "TRN2",
        target_bir_lowering=False,
        debug=False,
        enable_asserts=False,
        num_devices=1,
    )
    x = nc.dram_tensor("x", [C, H, W], f32, kind="ExternalInput").ap()
    d = nc.dram_tensor("d", [H, W], f32, kind="ExternalInput").ap()
    o = nc.dram_tensor("o", [C, Ho, Wo], f32, kind="ExternalOutput").ap()

    with TileContext(nc) as tc, ExitStack() as ctx:
        xp = ctx.enter_context(tc.tile_pool(name="xp", bufs=2))
        dp = ctx.enter_context(tc.tile_pool(name="dp", bufs=1))
        wp = ctx.enter_context(tc.tile_pool(name="wp", bufs=1))
        tp = ctx.enter_context(tc.tile_pool(name="tp", bufs=1))
        pp = ctx.enter_context(tc.tile_pool(name="pp", bufs=1))
        ap_ = ctx.enter_context(tc.tile_pool(name="ap", bufs=2))

        for _rep in range(repeat):
            # ---- depth tiles: DT[p, t, w] = d[4p+t, w]; D4[p, w] = d[4p+4, w]
            DT = dp.tile([PE, 4, W], f32, tag="DT")
            nc.sync.dma_start(
                out=DT[:], in_=d[0 : 4 * PE, :].rearrange("(p t) w -> p t w", t=4)
            )
            D4 = dp.tile([PO, W], f32, tag="D4")
            nc.sync.dma_start(out=D4[:], in_=d[4 : 4 * PO + 1 : 4, :])

            # window-center depth, both subs: dc[p, s, wo] = d[4p+2s+1, 2wo+1]
            dcm = DT[0:PE, 1:4:2, 1 : 1 + 2 * Wo : 2]  # [PE, 2, Wo]
            dce = DT[0:PE, 1, 1 : 1 + 2 * Wo : 2]  # [PE, Wo]
            dco = DT[0:PO, 3, 1 : 1 + 2 * Wo : 2]  # [PO, Wo]

            # ---- 8 non-center weight maps wm[p, s, wo] (normalized later)
            wm = {}
            for (i, j) in NC_TAPS:
                wt = wp.tile([PE, 2, Wo], f32, tag=f"w{i}{j}")
                if i < 2:
                    dv = DT[0:PE, i : i + 3 : 2, j : j + 2 * Wo : 2]
                    df = tp.tile([PE, 2, Wo], f32, tag="df")
                    nc.vector.tensor_sub(df[:], dcm, dv)
                    ab = tp.tile([PE, 2, Wo], f32, tag="ab")
                    nc.vector.scalar_tensor_tensor(
                        ab[:], df[:], -1.0, df[:], AluOp.mult, AluOp.max
                    )
                    nc.scalar.activation(wt[:], ab[:], Act.Exp, scale=-1.0)
                else:
                    # even sub from DT row t=2; odd sub from D4; pad rows -> 0
                    nc.vector.memzero(wt[:])
                    dfe = tp.tile([PE, Wo], f32, tag="dfe")
                    nc.vector.tensor_sub(dfe[:], dce, DT[0:PE, 2, j : j + 2 * Wo : 2])
                    abe = tp.tile([PE, Wo], f32, tag="abe")
                    nc.vector.scalar_tensor_tensor(
                        abe[:], dfe[:], -1.0, dfe[:], AluOp.mult, AluOp.max
                    )
                    nc.scalar.activation(wt[0:PE, 0, :], abe[:], Act.Exp, scale=-1.0)
                    dfo = tp.tile([PO, Wo], f32, tag="dfo")
                    nc.vector.tensor_sub(dfo[:], dco, D4[0:PO, j : j + 2 * Wo : 2])
                    abo = tp.tile([PO, Wo], f32, tag="abo")
                    nc.vector.scalar_tensor_tensor(
                        abo[:], dfo[:], -1.0, dfo[:], AluOp.mult, AluOp.max
                    )
                    nc.scalar.activation(wt[0:PO, 1, :], abo[:], Act.Exp, scale=-1.0)
                wm[(i, j)] = wt

            # ---- den = 1 + sum of the 8 maps; rden = 1/den
            ks = list(wm)
            s01 = tp.tile([PE, 2, Wo], f32, tag="s01")
            nc.vector.tensor_add(s01[:], wm[ks[0]][:], wm[ks[1]][:])
            s23 = tp.tile([PE, 2, Wo], f32, tag="s23")
            nc.vector.tensor_add(s23[:], wm[ks[2]][:], wm[ks[3]][:])
            s45 = tp.tile([PE, 2, Wo], f32, tag="s45")
            nc.vector.tensor_add(s45[:], wm[ks[4]][:], wm[ks[5]][:])
            s67 = tp.tile([PE, 2, Wo], f32, tag="s67")
            nc.vector.tensor_add(s67[:], wm[ks[6]][:], wm[ks[7]][:])
            nc.vector.tensor_add(s01[:], s01[:], s23[:])
            nc.vector.tensor_add(s45[:], s45[:], s67[:])
            nc.vector.tensor_add(s01[:], s01[:], s45[:])
            den = tp.tile([PE, 2, Wo], f32, tag="den")
            nc.vector.tensor_scalar_add(den[:], s01[:], 1.0)
            rden = wp.tile([PE, 2, Wo], f32, tag="rden")
            nc.vector.reciprocal(rden[:], den[:])
            # normalize in place; center weight becomes rden itself
            for wt in wm.values():
                nc.vector.tensor_mul(wt[:], wt[:], rden[:])

            if variant == "bf16":
                # convert the 9 normalized maps to padded bf16 tiles
                wmb = {}
                for (i, j) in NC_TAPS + [(1, 1)]:
                    src = rden if (i, j) == (1, 1) else wm[(i, j)]
                    wb = wp.tile([PE, 2, Wp], bf16, tag=f"wb{i}{j}")
                    nc.vector.memzero(wb[:])
                    nc.scalar.copy(wb[0:PE, :, 0:Wo], src[:])
                    wmb[(i, j)] = wb
                _main_bf16(nc, tc, xp, pp, ap_, x, o, wmb, C, G, W, Ho, Wo, Wp, PE, PO)
                continue

            # ---- main channel loop
            preXT = None
            if "nodma" in ablate:
                preXT = ap_.tile([PE, G, 5, W], f32, tag="preXT")
                nc.vector.memzero(preXT[:])
            for g in range(C // G):
                c0 = g * G
                if "nodma" in ablate:
                    XT = preXT
                elif "shiftmode" not in ablate:
                    # main load: rows 4p..4p+3 = one 8 KB DRAM read per (p,c);
                    # t=4 (row 4p+4) re-read from DRAM as its own clean DMA
                    XT = xp.tile([PE, G, 5, W], f32, tag="XT")
                    nc.sync.dma_start(
                        out=XT[0:PE, :, 0:4],
                        in_=x[c0 : c0 + G, 0 : 4 * PE, :].rearrange(
                            "c (p t) w -> p c (t w)", t=4
                        ),
                    )
                    nc.sync.dma_start(
                        out=XT[0:PO, :, 4],
                        in_=x[c0 : c0 + G, 4 : 4 * PO + 1 : 4, :].transpose(
                            [1, 0, 2]
                        ),
                    )
                    if PE > PO:  # t=4 pad for the last partition
                        nc.sync.dma_start(
                            out=XT[PO:PE, :, 4],
                            in_=x[c0 : c0 + G, 0:1, :].unsqueeze(0),
                        )
                else:
                    # layout [p, c, t, w]: per (p,c) the 4 rows 4p..4p+3 are
                    # one contiguous 8 KB DRAM read
                    XT = xp.tile([PE, G, 5, W], f32, tag="XT")
                    nc.sync.dma_start(
                        out=XT[0:PE, :, 0:4],
                        in_=x[c0 : c0 + G, 0 : 4 * PE, :].rearrange(
                            "c (p t) w -> p c (t w)", t=4
                        ),
                    )
                    # t=4 = row 4p+4 = next partition's t=0, via DMA shift
                    # (SWDGE ring so it doesn't block later loads on SP)
                    if "noshift" not in ablate:
                        nc.gpsimd.dma_start(
                            out=XT[0:PO, :, 4], in_=XT[1 : PO + 1, :, 0]
                        )
                        if PE > PO:  # last partition's odd sub never stored
                            nc.gpsimd.dma_start(
                                out=XT[PO:PE, :, 4], in_=XT[0:1, :, 0]
                            )

                # x tap views, merged over subs (sub stride = 2 slots in t)
                def xv_m(i, j):  # [PE, G, 2, Wo]
                    return XT[0:PE, :, i : i + 3 : 2, j : j + 2 * Wo : 2]

                def wv_m(wt):  # [PE, G, 2, Wo] broadcast over channels
                    return wt[0:PE].unsqueeze(1).broadcast_to([PE, G, 2, Wo])

                acc = ap_.tile([PE, G, 2, Wo], f32, tag="acc")
                if "nodve" in ablate:
                    nc.vector.memzero(acc[:])
                else:
                    # center tap: acc = x_center * rden
                    nc.vector.tensor_mul(acc[:], xv_m(1, 1), wv_m(rden))
                    for (i, j) in NC_TAPS:
                        p = pp.tile([PE, G, 2, Wo], f32, tag="pm")
                        nc.vector.tensor_mul(p[:], xv_m(i, j), wv_m(wm[(i, j)]))
                        nc.vector.tensor_add(acc[:], acc[:], p[:])

                if "nodma" in ablate or "nostore" in ablate:
                    continue
                # ---- store: rows 2p,2p+1 together -> 2040B-contiguous DRAM
                # descriptors (ACT HWDGE ring so waits don't block SP loads)
                nc.scalar.dma_start(
                    out=o[c0 : c0 + G, 0 : 2 * PO, :].rearrange(
                        "c (p s) w -> p c (s w)", s=2
                    ),
                    in_=acc[0:PO, :, :, :],
                )
                if Ho > 2 * PO:
                    nc.scalar.dma_start(  # last even row (ho = 2*PO)
                        out=o[c0 : c0 + G, 2 * PO : Ho, :].transpose([1, 0, 2]),
                        in_=acc[PO:PE, :, 0, :],
                    )

    nc.compile()
    return nc


@functools.lru_cache(maxsize=4)
def _compiled(key):
    C_, H_, W_, G, repeat, variant = key
    return build_kernel(C=C_, H=H_, W=W_, G=G, repeat=repeat, variant=variant)


def kernel(input, depth):
    """Full-io entry: input [8,64,512,512] f32, depth [8,1,512,512] f32."""
    from concourse import bass_utils

    input = np.ascontiguousarray(np.asarray(input), dtype=np.float32)
    depth = np.ascontiguousarray(np.asarray(depth), dtype=np.float32)
    N = input.shape[0]
    assert N == N_CORES and input.shape[1:] == (C, H, W)

    nc = _compiled((C, H, W, DEFAULT_G, 1, DEFAULT_VARIANT))
    in_maps = [{"x": input[n], "d": depth[n, 0]} for n in range(N)]
    res = bass_utils.run_bass_kernel_spmd(nc, in_maps, core_ids=list(range(N)))
    out = np.stack([r["o"] for r in res.results], axis=0)
    return out


if __name__ == "__main__":
    nc = build_kernel()
    print("built ok")



# revision 19
# speedup vs baseline: 2.7359x; 2.7359x over previous
"""Depth-weighted average pooling (3x3, stride 2) on 8 Trainium2 NeuronCores.

out[n,c,ho,wo] = sum_ij x[n,c,2ho+i,2wo+j] * w_ij / sum_ij w_ij
  w_ij = exp(-|d[n,2ho+1,2wo+1] - d[n,2ho+i,2wo+j]|)

Sharding: batch N=8, one image per core (data parallel, no halo).

Per-core layout ("rows mod 4" mapping): SBUF partition p holds input rows
4p..4p+3 (tile dim t) plus a re-read of row 4p+4 (T0'), so partition p
computes output rows 2p ("even sub", input rows 4p..4p+2) and 2p+1
("odd sub", rows 4p+2..4p+4).  All engine access patterns start at
partition 0 (hardware requires start partition in {0,32,64,96}).
Weights are computed in the same [p, sub, wo] layout, so they are direct
per-partition operands of the vector engine — no cross-partition
broadcast is ever needed.  Channels live in the free dimension.
"""

import os
import sys
import functools

import numpy as np

for _p in ("/opt/trn_rl_repo", "/opt/trn_rl_repo/concourse"):
    if os.path.isdir(_p) and _p not in sys.path:
        sys.path.insert(0, _p)

KH = KW = 3
SH = SW = 2
N_CORES = 8
C, H, W = 64, 512, 512

# defaults used by kernel() and the test harness
DEFAULT_VARIANT = "l2p"
DEFAULT_G = 4

# taps in order; (1,1) is the center
TAPS = [(i, j) for i in range(3) for j in range(3)]
NC_TAPS = [t for t in TAPS if t != (1, 1)]


def AP_load5(x, c0, G, P, W):
    """DRAM view for [p, c, 5, w] <- x[c, 4p+t, w]: 5*W contiguous per (p,c)."""
    import concourse.bass as bass

    HW_ = x.shape[1] * x.shape[2]
    return bass.AP(
        x.tensor,
        c0 * HW_,
        [[4 * W, P], [HW_, G], [1, 5 * W]],
    )


def _main_bf16(nc, tc, xp, pp, ap_, x, o, wmb, C, G, W, Ho, Wo, Wp, PE, PO, ablate=""):
    """bf16 main channel loop.

    Per group: SWDGE cast-DMA loads x rows as contiguous bf16; ScalarE
    de-interleaves each row into three 4B-aligned tap arrays
    [j0 | j1 | j2] (each Wp wide) so every vector op runs in the packed
    2x bf16 mode; the odd-sub i=2 operand is built by a partition-shift
    SBUF->SBUF DMA of the expanded tile.  Accumulation in bf16; the
    store casts back to fp32 in the DMA.
    """
    from concourse import mybir

    bf16 = mybir.dt.bfloat16
    f32 = mybir.dt.float32

    preXC = None
    if "nodma" in ablate:
        preXC = ap_.tile([PE, G, 5, W], bf16, tag="preXC")
        nc.vector.memzero(preXC[:])
    for g in range(C // G):
        c0 = g * G
        if "nodma" in ablate:
            XC = preXC
        else:
            # main load rows 4p..4p+3 (8 KB DRAM reads) + t=4 re-read,
            # cast fp32->bf16 in the DMA (SWDGE ring carries only loads)
            XC = xp.tile([PE, G, 5, W], bf16, tag="XC")
            nc.gpsimd.dma_start(
                out=XC[0:PE, :, 0:4],
                in_=x[c0 : c0 + G, 0 : 4 * PE, :].rearrange(
                    "c (p t) w -> p c (t w)", t=4
                ),
            )
            nc.gpsimd.dma_start(
                out=XC[0:PO, :, 4],
                in_=x[c0 : c0 + G, 4 : 4 * PO + 1 : 4, :].transpose([1, 0, 2]),
            )
            if PE > PO:  # t=4 pad for the last partition
                nc.gpsimd.dma_start(
                    out=XC[PO:PE, :, 4],
                    in_=x[c0 : c0 + G, 0:1, :].unsqueeze(0),
                )
        # expand rows t=0..4: XB[p,c,t] = [x[..,0::2] | x[..,1::2] | x[..,2::2] pad]
        XB = xp.tile([PE, G, 5, 3 * Wp], bf16, tag="XB")
        nc.scalar.copy(XB[0:PE, :, :, 0:Wp], XC[0:PE, :, :, 0 : 2 * Wp : 2])
        nc.scalar.copy(XB[0:PE, :, :, Wp : 2 * Wp], XC[0:PE, :, :, 1 : 2 * Wp : 2])
        nc.scalar.copy(
            XB[0:PE, :, :, 2 * Wp : 2 * Wp + Wo],
            XC[0:PE, :, :, 2 : 2 + 2 * Wo : 2],
        )
        # defined (finite) pad column for j2
        nc.scalar.copy(XB[0:PE, :, :, 3 * Wp - 1 : 3 * Wp], XC[0:PE, :, :, 0:1])

        def xv_m(i, j):  # [PE, G, 2, Wp]; sub stride = 2 slots in t
            return XB[0:PE, :, i : i + 3 : 2, j * Wp : (j + 1) * Wp]

        def wv_m(wb):
            return wb[0:PE].unsqueeze(1).broadcast_to([PE, G, 2, Wp])

        acc = ap_.tile([PE, G, 2, Wp], bf16, tag="acc")
        if "nodve" in ablate:
            nc.vector.memzero(acc[:])
        else:
            nc.vector.tensor_mul(acc[:], xv_m(1, 1), wv_m(wmb[(1, 1)]))
            for (i, j) in NC_TAPS:
                p = pp.tile([PE, G, 2, Wp], bf16, tag="pm")
                nc.vector.tensor_mul(p[:], xv_m(i, j), wv_m(wmb[(i, j)]))
                nc.vector.tensor_add(acc[:], acc[:], p[:])

        if "nodma" in ablate or "nostore" in ablate:
            continue
        # convert to fp32 on ScalarE, then merged 2-row store on the ACT ring
        OS = ap_.tile([PE, G, 2, Wo], f32, tag="OS")
        nc.scalar.copy(OS[:], acc[0:PE, :, :, 0:Wo])
        st = nc.gpsimd if "swstore" in ablate else nc.scalar
        st.dma_start(
            out=o[c0 : c0 + G, 0 : 2 * PO, :].rearrange(
                "c (p s) w -> p c (s w)", s=2
            ),
            in_=OS[0:PO, :, :, :],
        )
        if Ho > 2 * PO:
            st.dma_start(
                out=o[c0 : c0 + G, 2 * PO : Ho, :].transpose([1, 0, 2]),
                in_=OS[PO:PE, :, 0, :],
            )


def build_l2(C=C, H=H, W=W, repeat=1, ablate=""):
    """Big-descriptor layout: partition p = (c_lo, rb) holds a 16-input-row
    block (rows 16*rb .. 16*rb+16) of ONE channel; 128//NB channels per
    load tile.  Loads are one ~35 KB DRAM-contiguous descriptor per
    partition, stores one ~8 KB descriptor, vs ~10x 2-8 KB descriptors in
    the rows-mod-4 layout (HW DMA is descriptor-bound at ~0.8 us/desc).

    Weights are computed in the rows-mod-4 layout ([2, Wo] per partition,
    work spread over all 128 partitions), then redistributed to the block
    layout with 16 SBUF->SBUF DMAs (one per (c_lo, k) pair).

    Compute is bf16: ScalarE de-interleaves x rows into [even|odd] column
    arrays so the j=0/1 taps run in the packed 2x DVE mode; j=2 taps read
    the even array shifted one element (2-byte misaligned -> 1x mode).
    """
    from contextlib import ExitStack

    import concourse.bass as bass
    import concourse.bacc as bacc
    from concourse.tile import TileContext
    from concourse import mybir

    f32 = mybir.dt.float32
    bf16 = mybir.dt.bfloat16
    AluOp = mybir.AluOpType
    Act = mybir.ActivationFunctionType

    BR = 16                      # input rows per block
    OB = BR // 2                 # output rows per block
    NB = H // BR                 # row blocks
    CL = 128 // NB               # channel lanes (channels per tile)
    NT = C // CL                 # load tiles
    Ho = (H - KH) // SH + 1
    Wo = (W - KW) // SW + 1
    Wp = W // 2                  # padded output width (= Wo + 1)
    HW_ = H * W
    assert H % BR == 0 and 128 % NB == 0 and C % CL == 0
    PE = (Ho + 1) // 2           # rows-mod-4 prep partitions
    PO = Ho // 2
    assert PE == H // 4 and 2 * PE - 1 == Ho

    nc = bacc.Bacc(
        "TRN2",
        target_bir_lowering=False,
        debug=False,
        enable_asserts=False,
        num_devices=1,
    )
    x = nc.dram_tensor("x", [C, H, W], f32, kind="ExternalInput").ap()
    d = nc.dram_tensor("d", [H, W], f32, kind="ExternalInput").ap()
    o = nc.dram_tensor("o", [C, Ho, Wo], f32, kind="ExternalOutput").ap()

    # tap order; m = 3*i + j, center m=4
    with TileContext(nc) as tc, ExitStack() as ctx:
        dp = ctx.enter_context(tc.tile_pool(name="dp", bufs=1))
        wp = ctx.enter_context(tc.tile_pool(name="wp", bufs=1))
        tp = ctx.enter_context(tc.tile_pool(name="tp", bufs=1))
        xp = ctx.enter_context(tc.tile_pool(name="xp", bufs=2))
        bp = ctx.enter_context(tc.tile_pool(name="bp", bufs=2))
        ap_ = ctx.enter_context(tc.tile_pool(name="ap", bufs=2))
        pp = ctx.enter_context(tc.tile_pool(name="pp", bufs=1))

        KK = OB // 2  # source partitions per target block (= 4)
        for _rep in range(repeat):
            # ===== rows-mod-4 weight prep, in "q2" partition order ======
            # partition q2 = KK-phase k (q2 // NB) + block rb (q2 % NB)
            # holds the output-row pair ho = (2q, 2q+1), q = KK*rb + k.
            # This ordering makes the redistribution below read contiguous
            # partition ranges.  Depth rows for q2: 4q .. 4q+4 =
            # 16rb + 4k + (0..4).
            DT = dp.tile([PE, 4, W], f32, tag="DT")
            D4 = dp.tile([PO, W], f32, tag="D4")
            import concourse.bass as _bass

            for k in range(KK):
                nc.sync.dma_start(
                    out=DT[k * NB : (k + 1) * NB],
                    in_=_bass.AP(
                        d.tensor, 4 * k * W, [[4 * KK * W, NB], [W, 4], [1, W]]
                    ),
                )
                nb_k = NB if k < KK - 1 else NB - 1
                nc.sync.dma_start(
                    out=D4[k * NB : k * NB + nb_k],
                    in_=_bass.AP(
                        d.tensor,
                        (4 * k + 4) * W,
                        [[4 * KK * W, nb_k], [1, W]],
                    ),
                )

            dcm = DT[0:PE, 1:4:2, 1 : 1 + 2 * Wo : 2]  # [PE, 2, Wo]
            dce = DT[0:PE, 1, 1 : 1 + 2 * Wo : 2]
            dco = DT[0:PO, 3, 1 : 1 + 2 * Wo : 2]

            wm = {}
            for (i, j) in NC_TAPS:
                wt = wp.tile([PE, 2, Wo], f32, tag=f"w{i}{j}")
                if i < 2:
                    dv = DT[0:PE, i : i + 3 : 2, j : j + 2 * Wo : 2]
                    df = tp.tile([PE, 2, Wo], f32, tag="df")
                    nc.vector.tensor_sub(df[:], dcm, dv)
                    ab = tp.tile([PE, 2, Wo], f32, tag="ab")
                    nc.vector.scalar_tensor_tensor(
                        ab[:], df[:], -1.0, df[:], AluOp.mult, AluOp.max
                    )
                    nc.scalar.activation(wt[:], ab[:], Act.Exp, scale=-1.0)
                else:
                    nc.vector.memzero(wt[:])
                    dfe = tp.tile([PE, Wo], f32, tag="dfe")
                    nc.vector.tensor_sub(dfe[:], dce, DT[0:PE, 2, j : j + 2 * Wo : 2])
                    abe = tp.tile([PE, Wo], f32, tag="abe")
                    nc.vector.scalar_tensor_tensor(
                        abe[:], dfe[:], -1.0, dfe[:], AluOp.mult, AluOp.max
                    )
                    nc.scalar.activation(wt[0:PE, 0, :], abe[:], Act.Exp, scale=-1.0)
                    dfo = tp.tile([PO, Wo], f32, tag="dfo")
                    nc.vector.tensor_sub(dfo[:], dco, D4[0:PO, j : j + 2 * Wo : 2])
                    abo = tp.tile([PO, Wo], f32, tag="abo")
                    nc.vector.scalar_tensor_tensor(
                        abo[:], dfo[:], -1.0, dfo[:], AluOp.mult, AluOp.max
                    )
                    nc.scalar.activation(wt[0:PO, 1, :], abo[:], Act.Exp, scale=-1.0)
                wm[(i, j)] = wt

            ks = list(wm)
            s01 = tp.tile([PE, 2, Wo], f32, tag="s01")
            nc.vector.tensor_add(s01[:], wm[ks[0]][:], wm[ks[1]][:])
            s23 = tp.tile([PE, 2, Wo], f32, tag="s23")
            nc.vector.tensor_add(s23[:], wm[ks[2]][:], wm[ks[3]][:])
            s45 = tp.tile([PE, 2, Wo], f32, tag="s45")
            nc.vector.tensor_add(s45[:], wm[ks[4]][:], wm[ks[5]][:])
            s67 = tp.tile([PE, 2, Wo], f32, tag="s67")
            nc.vector.tensor_add(s67[:], wm[ks[6]][:], wm[ks[7]][:])
            nc.vector.tensor_add(s01[:], s01[:], s23[:])
            nc.vector.tensor_add(s45[:], s45[:], s67[:])
            nc.vector.tensor_add(s01[:], s01[:], s45[:])
            den = tp.tile([PE, 2, Wo], f32, tag="den")
            nc.vector.tensor_scalar_add(den[:], s01[:], 1.0)
            rden = wp.tile([PE, 2, Wo], f32, tag="rden")
            nc.vector.reciprocal(rden[:], den[:])
            for wt in wm.values():
                nc.vector.tensor_mul(wt[:], wt[:], rden[:])

            # pack normalized maps as bf16 into Wold[q, m, s, wp]
            Wold = wp.tile([PE, 9, 2, Wp], bf16, tag="Wold")
            nc.vector.memzero(Wold[:])
            for (i, j) in TAPS:
                m = 3 * i + j
                src = rden if (i, j) == (1, 1) else wm[(i, j)]
                nc.scalar.copy(Wold[0:PE, m, :, 0:Wo], src[:])

            # ============ redistribute: W9L[p, k, m, s, wp] ==============
            # target p = (c_lo, rb) needs source pairs q2 = k*NB + rb,
            # k = 0..KK-1 -- contiguous partition ranges on both sides
            W9 = wp.tile([128, KK, 9, 2, Wp], bf16, tag="W9")
            for c_lo in range(CL):
                for k in range(KK):
                    nc.sync.dma_start(
                        out=W9[c_lo * NB : (c_lo + 1) * NB, k],
                        in_=Wold[k * NB : (k + 1) * NB],
                    )

            if "nomain" in ablate:
                continue

            # ===================== main tile loop ========================
            for t in range(NT):
                c0 = t * CL
                # 16-row main load: one non-overlapping 32KB-read descriptor
                # per partition (overlapping windows serialize the SDMA
                # engines ~17x).  Halo row 16*rb+16 comes as a second DMA --
                # it duplicates the next block's first row, but the overlap
                # is across dma_starts, which is fine.
                XC = xp.tile([128, BR + 1, W], bf16, tag="XC")
                nc.gpsimd.dma_start(
                    out=XC[:, 0:BR],
                    in_=bass.AP(x.tensor, c0 * HW_, [[BR * W, 128], [1, BR * W]]),
                )
                nhalo = 128 if (c0 + CL) * HW_ + W <= C * HW_ else 127
                nc.gpsimd.dma_start(
                    out=XC[0:nhalo, BR],
                    in_=bass.AP(
                        x.tensor, c0 * HW_ + BR * W, [[BR * W, nhalo], [1, W]]
                    ),
                )
                if nhalo < 128:
                    # final tile: partition 127's halo would run off x;
                    # duplicate a finite row instead (never used)
                    nc.gpsimd.dma_start(
                        out=XC[127:128, BR],
                        in_=x[C - 1 : C, H - BR, :],
                    )

                # de-interleave columns: XB[., r, 0, :] = even, [., r, 1, :] = odd
                XB = bp.tile([128, BR + 1, 2, Wp], bf16, tag="XB")
                nc.scalar.copy(XB[:, :, 0, :], XC[:, :, 0 : W : 2])
                nc.scalar.copy(XB[:, :, 1, :], XC[:, :, 1 : W : 2])

                def xv(i, j):  # [128, KK, 2, Wp]: output row r=2k+s -> input 2r+i
                    if j < 2:
                        v = XB[:, i : i + 2 * OB - 1 : 2, j, :]
                    else:
                        # even array shifted one element (2B-misaligned, 1x)
                        v = (
                            XB[:, i : i + 2 * OB - 1 : 2]
                            .rearrange("p r two w -> p r (two w)")[:, :, 1 : Wp + 1]
                        )
                    return v.rearrange("p (a b) w -> p a b w", b=2)

                def wv(i, j):  # [128, KK, 2, Wp]
                    return W9[:, :, 3 * i + j]

                acc = ap_.tile([128, KK, 2, Wp], bf16, tag="acc")
                if "nodve" in ablate:
                    nc.vector.memzero(acc[:])
                else:
                    # j=2 tap muls run 1x on DVE (2B-misaligned operand), so
                    # they are the cheapest to push to GPSIMD
                    mul_eng = nc.gpsimd if "pool2" in ablate else nc.vector
                    nc.vector.tensor_mul(acc[:], xv(1, 1), wv(1, 1))
                    for (i, j) in NC_TAPS:
                        pm = pp.tile([128, KK, 2, Wp], bf16, tag=f"pm{j==2}")
                        eng = mul_eng if j == 2 else nc.vector
                        eng.tensor_mul(pm[:], xv(i, j), wv(i, j))
                        nc.vector.tensor_add(acc[:], acc[:], pm[:])

                # pack to fp32 [OB, Wo] and store one 8KB desc per partition
                OS = ap_.tile([128, OB, Wo], f32, tag="OS")
                accr = acc[:].rearrange("p a b w -> p (a b) w")
                nc.scalar.copy(OS[:], accr[:, :, 0:Wo])
                if "nostore" in ablate:
                    continue
                HoWo = Ho * Wo
                st = nc.gpsimd if "swst" in ablate else nc.scalar
                for c_lo in range(CL):
                    cc = c0 + c_lo
                    st.dma_start(
                        out=bass.AP(
                            o.tensor,
                            cc * HoWo,
                            [[OB * Wo, NB - 1], [1, OB * Wo]],
                        ),
                        in_=OS[c_lo * NB : (c_lo + 1) * NB - 1],
                    )
                    # last block stores OB-1 rows (ho = Ho-1 ends the image)
                    st.dma_start(
                        out=o[cc : cc + 1, (NB - 1) * OB : Ho, :],
                        in_=OS[(c_lo + 1) * NB - 1 : (c_lo + 1) * NB, 0 : OB - 1],
                    )

    nc.compile()
    return nc


def build_kernel(C=C, H=H, W=W, G=4, repeat=1, variant="fp32", ablate=""):
    """Single-core Bass program: x[C,H,W], d[H,W] -> o[C,Ho,Wo].

    ablate: "nodve" skips the main-loop vector ops; "nodma" skips the
    x loads (compute reads whatever is resident).  For A/B timing only.
    """
    from contextlib import ExitStack

    import concourse.bacc as bacc
    from concourse.tile import TileContext
    from concourse import mybir

    if variant in ("l2", "l2p"):
        if variant == "l2p" and "pool2" not in ablate:
            ablate = (ablate + " pool2").strip()
        return build_l2(C=C, H=H, W=W, repeat=repeat, ablate=ablate)

    f32 = mybir.dt.float32
    bf16 = mybir.dt.bfloat16
    AluOp = mybir.AluOpType
    Act = mybir.ActivationFunctionType
    Wp = W // 2  # padded output-width for bf16 tiles (= Wo+1, even)

    Ho = (H - KH) // SH + 1
    Wo = (W - KW) // SW + 1
    assert C % G == 0 and H % 4 == 0
    PE = (Ho + 1) // 2  # partitions carrying an even-sub output row
    PO = Ho // 2  # partitions carrying an odd-sub output row
    assert PE <= 128 and Ho == 2 * PE - 1

    nc = bacc.Bacc(
        "TRN2",
        target_bir_lowering=False,
        debug=False,
        enable_asserts=False,
        num_devices=1,
    )
    x = nc.dram_tensor("x", [C, H, W], f32, kind="ExternalInput").ap()
    d = nc.dram_tensor("d", [H, W], f32, kind="ExternalInput").ap()
    o = nc.dram_tensor("o", [C, Ho, Wo], f32, kind="ExternalOutput").ap()

    with TileContext(nc) as tc, ExitStack() as ctx:
        xp = ctx.enter_context(tc.tile_pool(name="xp", bufs=2))
        dp = ctx.enter_context(tc.tile_pool(name="dp", bufs=1))
        wp = ctx.enter_context(tc.tile_pool(name="wp", bufs=1))
        tp = ctx.enter_context(tc.tile_pool(name="tp", bufs=1))
        pp = ctx.enter_context(tc.tile_pool(name="pp", bufs=1))
        ap_ = ctx.enter_context(tc.tile_pool(name="ap", bufs=2))

        for _rep in range(repeat):
            # ---- depth tiles: DT[p, t, w] = d[4p+t, w]; D4[p, w] = d[4p+4, w]
            DT = dp.tile([PE, 4, W], f32, tag="DT")
            nc.sync.dma_start(
                out=DT[:], in_=d[0 : 4 * PE, :].rearrange("(p t) w -> p t w", t=4)
            )
            D4 = dp.tile([PO, W], f32, tag="D4")
            nc.sync.dma_start(out=D4[:], in_=d[4 : 4 * PO + 1 : 4, :])

            # window-center depth, both subs: dc[p, s, wo] = d[4p+2s+1, 2wo+1]
            dcm = DT[0:PE, 1:4:2, 1 : 1 + 2 * Wo : 2]  # [PE, 2, Wo]
            dce = DT[0:PE, 1, 1 : 1 + 2 * Wo : 2]  # [PE, Wo]
            dco = DT[0:PO, 3, 1 : 1 + 2 * Wo : 2]  # [PO, Wo]

            # ---- 8 non-center weight maps wm[p, s, wo] (normalized later)
            wm = {}
            for (i, j) in NC_TAPS:
                wt = wp.tile([PE, 2, Wo], f32, tag=f"w{i}{j}")
                if i < 2:
                    dv = DT[0:PE, i : i + 3 : 2, j : j + 2 * Wo : 2]
                    df = tp.tile([PE, 2, Wo], f32, tag="df")
                    nc.vector.tensor_sub(df[:], dcm, dv)
                    ab = tp.tile([PE, 2, Wo], f32, tag="ab")
                    nc.vector.scalar_tensor_tensor(
                        ab[:], df[:], -1.0, df[:], AluOp.mult, AluOp.max
                    )
                    nc.scalar.activation(wt[:], ab[:], Act.Exp, scale=-1.0)
                else:
                    # even sub from DT row t=2; odd sub from D4; pad rows -> 0
                    nc.vector.memzero(wt[:])
                    dfe = tp.tile([PE, Wo], f32, tag="dfe")
                    nc.vector.tensor_sub(dfe[:], dce, DT[0:PE, 2, j : j + 2 * Wo : 2])
                    abe = tp.tile([PE, Wo], f32, tag="abe")
                    nc.vector.scalar_tensor_tensor(
                        abe[:], dfe[:], -1.0, dfe[:], AluOp.mult, AluOp.max
                    )
                    nc.scalar.activation(wt[0:PE, 0, :], abe[:], Act.Exp, scale=-1.0)
                    dfo = tp.tile([PO, Wo], f32, tag="dfo")
                    nc.vector.tensor_sub(dfo[:], dco, D4[0:PO, j : j + 2 * Wo : 2])
                    abo = tp.tile([PO, Wo], f32, tag="abo")
                    nc.vector.scalar_tensor_tensor(
                        abo[:], dfo[:], -1.0, dfo[:], AluOp.mult, AluOp.max
                    )
                    nc.scalar.activation(wt[0:PO, 1, :], abo[:], Act.Exp, scale=-1.0)
                wm[(i, j)] = wt

            # ---- den = 1 + sum of the 8 maps; rden = 1/den
            ks = list(wm)
            s01 = tp.tile([PE, 2, Wo], f32, tag="s01")
            nc.vector.tensor_add(s01[:], wm[ks[0]][:], wm[ks[1]][:])
            s23 = tp.tile([PE, 2, Wo], f32, tag="s23")
            nc.vector.tensor_add(s23[:], wm[ks[2]][:], wm[ks[3]][:])
            s45 = tp.tile([PE, 2, Wo], f32, tag="s45")
            nc.vector.tensor_add(s45[:], wm[ks[4]][:], wm[ks[5]][:])
            s67 = tp.tile([PE, 2, Wo], f32, tag="s67")
            nc.vector.tensor_add(s67[:], wm[ks[6]][:], wm[ks[7]][:])
            nc.vector.tensor_add(s01[:], s01[:], s23[:])
            nc.vector.tensor_add(s45[:], s45[:], s67[:])
            nc.vector.tensor_add(s01[:], s01[:], s45[:])
            den = tp.tile([PE, 2, Wo], f32, tag="den")
            nc.vector.tensor_scalar_add(den[:], s01[:], 1.0)
            rden = wp.tile([PE, 2, Wo], f32, tag="rden")
            nc.vector.reciprocal(rden[:], den[:])
            # normalize in place; center weight becomes rden itself
            for wt in wm.values():
                nc.vector.tensor_mul(wt[:], wt[:], rden[:])

            if variant == "bf16":
                # convert the 9 normalized maps to padded bf16 tiles
                wmb = {}
                for (i, j) in NC_TAPS + [(1, 1)]:
                    src = rden if (i, j) == (1, 1) else wm[(i, j)]
                    wb = wp.tile([PE, 2, Wp], bf16, tag=f"wb{i}{j}")
                    nc.vector.memzero(wb[:])
                    nc.scalar.copy(wb[0:PE, :, 0:Wo], src[:])
                    wmb[(i, j)] = wb
                _main_bf16(
                    nc, tc, xp, pp, ap_, x, o, wmb, C, G, W, Ho, Wo, Wp, PE, PO,
                    ablate=ablate,
                )
                continue

            # ---- main channel loop
            preXT = None
            if "nodma" in ablate:
                preXT = ap_.tile([PE, G, 5, W], f32, tag="preXT")
                nc.vector.memzero(preXT[:])
            for g in range(C // G):
                c0 = g * G
                if "nodma" in ablate:
                    XT = preXT
                elif "load5" in ablate:
                    # single load: rows 4p..4p+4 as one 10 KB descriptor per
                    # (p, c); last partition loaded separately (row 4p+4
                    # would run off the end of x)
                    XT = xp.tile([PE, G, 5, W], f32, tag="XT")
                    eng = nc.gpsimd if "swdge" in ablate else nc.sync
                    eng.dma_start(
                        out=XT[0 : PE - 1, :, 0:5],
                        in_=AP_load5(x, c0, G, PE - 1, W),
                    )
                    eng.dma_start(
                        out=XT[PE - 1 : PE, :, 0:4],
                        in_=x[c0 : c0 + G, 4 * (PE - 1) : 4 * PE, :]
                        .rearrange("c (p t) w -> p c (t w)", t=4),
                    )
                    eng.dma_start(
                        out=XT[PE - 1 : PE, :, 4],
                        in_=x[c0 : c0 + G, 0:1, :].unsqueeze(0),
                    )
                elif "swdge" in ablate:
                    # same AP structure as the default path, but on the
                    # SWDGE (gpsimd) ring
                    XT = xp.tile([PE, G, 5, W], f32, tag="XT")
                    nc.gpsimd.dma_start(
                        out=XT[0:PE, :, 0:4],
                        in_=x[c0 : c0 + G, 0 : 4 * PE, :].rearrange(
                            "c (p t) w -> p c (t w)", t=4
                        ),
                    )
                    nc.gpsimd.dma_start(
                        out=XT[0:PO, :, 4],
                        in_=x[c0 : c0 + G, 4 : 4 * PO + 1 : 4, :].transpose(
                            [1, 0, 2]
                        ),
                    )
                    if PE > PO:
                        nc.gpsimd.dma_start(
                            out=XT[PO:PE, :, 4],
                            in_=x[c0 : c0 + G, 0:1, :].unsqueeze(0),
                        )
                elif "shiftmode" not in ablate:
                    # main load: rows 4p..4p+3 = one 8 KB DRAM read per (p,c);
                    # t=4 (row 4p+4) re-read from DRAM as its own clean DMA
                    XT = xp.tile([PE, G, 5, W], f32, tag="XT")
                    nc.sync.dma_start(
                        out=XT[0:PE, :, 0:4],
                        in_=x[c0 : c0 + G, 0 : 4 * PE, :].rearrange(
                            "c (p t) w -> p c (t w)", t=4
                        ),
                    )
                    nc.sync.dma_start(
                        out=XT[0:PO, :, 4],
                        in_=x[c0 : c0 + G, 4 : 4 * PO + 1 : 4, :].transpose(
                            [1, 0, 2]
                        ),
                    )
                    if PE > PO:  # t=4 pad for the last partition
                        nc.sync.dma_start(
                            out=XT[PO:PE, :, 4],
                            in_=x[c0 : c0 + G, 0:1, :].unsqueeze(0),
                        )
                else:
                    # layout [p, c, t, w]: per (p,c) the 4 rows 4p..4p+3 are
                    # one contiguous 8 KB DRAM read
                    XT = xp.tile([PE, G, 5, W], f32, tag="XT")
                    nc.sync.dma_start(
                        out=XT[0:PE, :, 0:4],
                        in_=x[c0 : c0 + G, 0 : 4 * PE, :].rearrange(
                            "c (p t) w -> p c (t w)", t=4
                        ),
                    )
                    # t=4 = row 4p+4 = next partition's t=0, via DMA shift
                    # (SWDGE ring so it doesn't block later loads on SP)
                    if "noshift" not in ablate:
                        nc.gpsimd.dma_start(
                            out=XT[0:PO, :, 4], in_=XT[1 : PO + 1, :, 0]
                        )
                        if PE > PO:  # last partition's odd sub never stored
                            nc.gpsimd.dma_start(
                                out=XT[PO:PE, :, 4], in_=XT[0:1, :, 0]
                            )

                # x tap views, merged over subs (sub stride = 2 slots in t)
                def xv_m(i, j):  # [PE, G, 2, Wo]
                    return XT[0:PE, :, i : i + 3 : 2, j : j + 2 * Wo : 2]

                def wv_m(wt):  # [PE, G, 2, Wo] broadcast over channels
                    return wt[0:PE].unsqueeze(1).broadcast_to([PE, G, 2, Wo])

                acc = ap_.tile([PE, G, 2, Wo], f32, tag="acc")
                if "nodve" in ablate:
                    nc.vector.memzero(acc[:])
                else:
                    # center tap: acc = x_center * rden
                    nc.vector.tensor_mul(acc[:], xv_m(1, 1), wv_m(rden))
                    for (i, j) in NC_TAPS:
                        p = pp.tile([PE, G, 2, Wo], f32, tag="pm")
                        nc.vector.tensor_mul(p[:], xv_m(i, j), wv_m(wm[(i, j)]))
                        nc.vector.tensor_add(acc[:], acc[:], p[:])

                if "nodma" in ablate or "nostore" in ablate:
                    continue
                # ---- store: rows 2p,2p+1 together -> 2040B-contiguous DRAM
                # descriptors (ACT HWDGE ring so waits don't block SP loads)
                st = nc.gpsimd if "swstore" in ablate else nc.scalar
                st.dma_start(
                    out=o[c0 : c0 + G, 0 : 2 * PO, :].rearrange(
                        "c (p s) w -> p c (s w)", s=2
                    ),
                    in_=acc[0:PO, :, :, :],
                )
                if Ho > 2 * PO:
                    st.dma_start(  # last even row (ho = 2*PO)
                        out=o[c0 : c0 + G, 2 * PO : Ho, :].transpose([1, 0, 2]),
                        in_=acc[PO:PE, :, 0, :],
                    )

    nc.compile()
    return nc


@functools.lru_cache(maxsize=4)
def _compiled(key):
    C_, H_, W_, G, repeat, variant = key
    return build_kernel(C=C_, H=H_, W=W_, G=G, repeat=repeat, variant=variant)


def kernel(input, depth):
    """Full-io entry: input [8,64,512,512] f32, depth [8,1,512,512] f32."""
    from concourse import bass_utils

    input = np.ascontiguousarray(np.asarray(input), dtype=np.float32)
    depth = np.ascontiguousarray(np.asarray(depth), dtype=np.float32)
    N = input.shape[0]
    assert N == N_CORES and input.shape[1:] == (C, H, W)

    variant = os.environ.get("KERNEL_VARIANT", DEFAULT_VARIANT)
    nc = _compiled((C, H, W, DEFAULT_G, 1, variant))
    in_maps = [{"x": input[n], "d": depth[n, 0]} for n in range(N)]
    res = bass_utils.run_bass_kernel_spmd(nc, in_maps, core_ids=list(range(N)))
    out = np.stack([r["o"] for r in res.results], axis=0)
    return out


if __name__ == "__main__":
    nc = build_kernel()
    print("built ok")

